# revision 1
# baseline (speedup 1.0000x reference)
"""GraphSAGE-mean GNN (3 layers + classifier) on 8 Trainium2 NeuronCores.

Strategy (data-parallel over nodes, sharded by dst):
  - Nodes padded 50000 -> 50176, degree-sorted and dealt round-robin over the
    8 cores so every core sees a near-identical degree profile (SPMD: one
    compiled program for all cores). Per core: 6272 nodes = 49 tiles of 128.
  - Neighbor gather uses the custom Q7 dma_gather instruction (int16 indices,
    4 SWDGE queues). Since indices are signed int16 (< 32768), the gather
    table is addressed through two windows: A = rows [0, 32767] (sources on
    cores 0-4) and B = rows [17409, 50175] (sources on cores 5-7). Zero-slot
    padding points at dummy-node rows which are explicitly zeroed.
  - Gathered neighbor blocks [128 nodes x 128 feat] are accumulated on the
    tensor engine (identity matmuls into PSUM) to keep the vector engine off
    the SBUF port that SWDGE descriptor generation needs.
  - deg_inv scaling + PSUM->SBUF copies run on the scalar (ACT) engine.
  - Dense layer matmuls run feature-major: out = lhsT.T @ rhs with the weight
    as lhsT and h^T as rhs. Aggregates are transposed per-tile via PE.
  - Node features for the next layer's gather are re-transposed to node-major
    fp16, DMA'd to DRAM and AllGathered across the 8 cores.
"""

import os
import numpy as np

P = 128
NC = 8
N = 50000
E = 800000
NPAD = 50176
SH = 6272  # nodes per core
TILES = 49
H = 128
NLAYERS = 3
ODIM = 40
ACUT = 31360  # sources with new id < ACUT use window A (cores 0-4)
B0 = 17409  # window B base row; idx = id - B0 (<= 32766 for id < 50176)
AZERO = ACUT - 1  # core 4's last node: a dummy (zeroed) row, < 32768
BZERO = NPAD - 1 - B0  # core 7's last node as B-window index
NQ = int(os.environ.get('GNN_NQ', '1'))
DENSE_BLOCKS = [(i * 512, 512) for i in range(12)] + [(6144, 128)]
N_DUMMY = SH - N // NC  # 22 dummy nodes per core, at the tail of tile 48

last_results = None


def _prep(edge_index):
    """Host-side graph preprocessing: permutation, per-tile neighbor slots."""
    src = edge_index[0].astype(np.int64)
    dst = edge_index[1].astype(np.int64)
    deg = np.bincount(dst, minlength=N).astype(np.float32)
    dinv = 1.0 / np.maximum(deg, 1.0)

    # pass 1: global degree sort, deal round-robin -> core assignment
    order = np.argsort(-deg, kind="stable")  # rank -> orig node
    core_of = np.empty(N, np.int64)
    core_of[order] = np.arange(N) % NC

    # pass 2: within-core order by (degA, degB) desc; degA counts in-edges
    # whose source core is in 0..4 (window A)
    srcA = core_of[src] < 5
    degA = np.bincount(dst[srcA], minlength=N).astype(np.int64)
    degB = np.bincount(dst[~srcA], minlength=N).astype(np.int64)

    newid = np.empty(N, np.int64)
    for c in range(NC):
        nodes = np.flatnonzero(core_of == c)
        key = (degA[nodes] // 8) * 1000000 + (degB[nodes] // 2) * 1000 \
            + degA[nodes] % 8 * 10 + degB[nodes] % 2
        nodes = nodes[np.argsort(-key, kind="stable")]
        newid[nodes] = c * SH + np.arange(len(nodes))  # dummies occupy the tail

    ns = newid[src]
    nd = newid[dst]
    isA = ns < ACUT

    # per-(core, tile) block counts; uniform across cores for SPMD
    def build(mask, zero_idx, rebase):
        ns_h, nd_h = ns[mask], nd[mask]
        o = np.argsort(nd_h, kind="stable")
        ns_h, nd_h = ns_h[o], nd_h[o]
        # cumcount within each dst node
        n_edges = len(nd_h)
        if n_edges == 0:
            return np.zeros(NC * NPAD // P // NC, np.int64), None, None
        firsts = np.r_[0, np.flatnonzero(np.diff(nd_h)) + 1]
        grp = np.zeros(n_edges, np.int64)
        grp[firsts] = 1
        grp = np.cumsum(grp) - 1
        d_in_node = np.arange(n_edges) - firsts[grp]
        cnt = np.bincount(nd_h, minlength=NC * SH)  # per new-node count
        cnt_t = cnt.reshape(NC, TILES, P)
        D_t = cnt_t.max(axis=2).max(axis=0)  # [TILES], max over cores
        base = np.r_[0, np.cumsum(D_t * P)]  # slot base per tile
        CA = int(base[-1])  # slots per core
        arr16 = np.full((NC, 16, CA // 16), zero_idx, np.int64)
        c_h = nd_h // SH
        pos = nd_h % SH
        t_h = pos // P
        p_h = pos % P
        flat = base[t_h] + d_in_node * P + p_h
        val = ns_h - rebase
        arr16[c_h, flat % 16, flat // 16] = val
        return D_t, base, arr16

    DA, baseA, arrA = build(isA, AZERO, 0)
    DB, baseB, arrB = build(~isA, BZERO, B0)

    inv = np.empty(N, np.int64)  # only defined for real nodes
    inv = newid[np.arange(N)]  # orig -> new
    return dict(
        dinv=dinv, newid=inv, DA=DA, DB=DB, baseA=baseA, baseB=baseB,
        arrA=arrA, arrB=arrB,
    )



def _patch_swdge_lane_by_queue():
    """Bind DMASW sem lanes to SWDGE queues (2 lanes per queue) so multi-queue
    SWDGE keeps the per-queue semaphore invariant the ucode reclaim needs.
    Returns an undo function."""
    import concourse.tile_sem_assignment as tsa
    import concourse.mybir as mybir
    from concourse import bass_isa
    from concourse.tile_scheduler import DMAInst

    orig = tsa.TileClockTick._assign_tick

    def patched(self, inst):
        try:
            is_pool_dma = (
                isinstance(inst, DMAInst)
                and inst.engine == mybir.EngineType.Pool
                and not isinstance(inst, bass_isa.UserSyncedRemoteDMADescs)
            )
        except Exception:
            is_pool_dma = False
        if is_pool_dma:
            q = int(getattr(inst, "queue_num", 0) or 0)
            counters = getattr(self, "_queue_lane_ctr", None)
            if counters is None:
                counters = {}
                self._queue_lane_ctr = counters
            k = counters.get(q, 0)
            counters[q] = k + 1
            lanes_per_q = max(1, self.swdge_sem_count // NQ)
            self.next_sw_dma_idx = (q * lanes_per_q + k % lanes_per_q) % (
                self.swdge_sem_count
            )
        return orig(self, inst)

    tsa.TileClockTick._assign_tick = patched

    def undo():
        tsa.TileClockTick._assign_tick = orig

    return undo


def _build_program(DA, DB, baseA, baseB, CA, CB):
    import concourse.bass as bass
    import concourse.bacc as bacc
    import concourse.tile as tile
    import concourse.mybir as mybir

    f32 = mybir.dt.float32
    f16 = mybir.dt.float16
    i16 = mybir.dt.int16
    AF = mybir.ActivationFunctionType

    nc = bacc.Bacc("TRN2", target_bir_lowering=False, debug=False,
                   num_devices=NC, num_swdge_queues=NQ)

    # ---- I/O ----
    xT = nc.dram_tensor("xT", [P, SH], f32, kind="ExternalInput")
    idxA = nc.dram_tensor("idxA", [P, CA // 16], i16, kind="ExternalInput")
    idxB = nc.dram_tensor("idxB", [P, CB // 16], i16, kind="ExternalInput")
    dinv_in = nc.dram_tensor("dinv", [P, TILES], f32, kind="ExternalInput")
    w_in = nc.dram_tensor("w_in", [H, H], f32, kind="ExternalInput")
    b_in = nc.dram_tensor("b_in", [H, 1], f32, kind="ExternalInput")
    wl_in = nc.dram_tensor("wl", [H, NLAYERS * H], f32, kind="ExternalInput")
    wr_in = nc.dram_tensor("wr", [H, NLAYERS * H], f16, kind="ExternalInput")
    bl_in = nc.dram_tensor("bl", [H, NLAYERS], f32, kind="ExternalInput")
    wc1_in = nc.dram_tensor("wc1", [H, H], f32, kind="ExternalInput")
    bc1_in = nc.dram_tensor("bc1", [H, 1], f32, kind="ExternalInput")
    wc2_in = nc.dram_tensor("wc2", [H, ODIM], f32, kind="ExternalInput")
    bc2_in = nc.dram_tensor("bc2", [ODIM, 1], f32, kind="ExternalInput")
    ident_in = nc.dram_tensor("ident", [P, P], f16, kind="ExternalInput")
    dmask_in = nc.dram_tensor("dmask", [P, 1], f32, kind="ExternalInput")
    outT = nc.dram_tensor("outT", [ODIM, SH], f32, kind="ExternalOutput")

    with tile.TileContext(nc) as tc:
        with (
            tc.tile_pool(name="consts", bufs=1) as consts,
            tc.tile_pool(name="big", bufs=1) as big,
            tc.tile_pool(name="hT", bufs=2) as hTp,
            tc.tile_pool(name="aggTp", bufs=1) as aggTp,
            tc.tile_pool(name="hnm", bufs=1) as hnmp,
            tc.tile_pool(name="gA", bufs=3) as gAp,
            tc.tile_pool(name="gB", bufs=3) as gBp,
            tc.tile_pool(name="small", bufs=4) as small,
            tc.tile_pool(name="psagg", bufs=3, space="PSUM") as psagg,
            tc.tile_pool(name="pstp", bufs=2, space="PSUM") as pstp,
            tc.tile_pool(name="psz", bufs=2, space="PSUM") as psz,
            tc.tile_pool(name="dram", bufs=1, space="DRAM") as dram,
        ):
            # ---- load constants ----
            idxA_sb = consts.tile([P, CA // 16], i16)
            nc.sync.dma_start(out=idxA_sb[:], in_=idxA[:])
            idxB_sb = consts.tile([P, CB // 16], i16)
            nc.sync.dma_start(out=idxB_sb[:], in_=idxB[:])
            dinv_sb = consts.tile([P, TILES], f32)
            nc.sync.dma_start(out=dinv_sb[:], in_=dinv_in[:])
            w_in_sb = consts.tile([H, H], f32)
            nc.sync.dma_start(out=w_in_sb[:], in_=w_in[:])
            b_in_sb = consts.tile([H, 1], f32)
            nc.sync.dma_start(out=b_in_sb[:], in_=b_in[:])
            wl_sb = consts.tile([H, NLAYERS * H], f32)
            nc.sync.dma_start(out=wl_sb[:], in_=wl_in[:])
            wr_sb = consts.tile([H, NLAYERS * H], f16)
            nc.sync.dma_start(out=wr_sb[:], in_=wr_in[:])
            bl_sb = consts.tile([H, NLAYERS], f32)
            nc.sync.dma_start(out=bl_sb[:], in_=bl_in[:])
            wc1_sb = consts.tile([H, H], f32)
            nc.sync.dma_start(out=wc1_sb[:], in_=wc1_in[:])
            bc1_sb = consts.tile([H, 1], f32)
            nc.sync.dma_start(out=bc1_sb[:], in_=bc1_in[:])
            wc2_sb = consts.tile([H, ODIM], f32)
            nc.sync.dma_start(out=wc2_sb[:], in_=wc2_in[:])
            bc2_sb = consts.tile([ODIM, 1], f32)
            nc.sync.dma_start(out=bc2_sb[:], in_=bc2_in[:])
            ident_sb = consts.tile([P, P], f16)
            nc.sync.dma_start(out=ident_sb[:], in_=ident_in[:])
            dmask_sb = consts.tile([P, 1], f32)
            nc.sync.dma_start(out=dmask_sb[:], in_=dmask_in[:])
            xsum = big.tile([P, SH], f32)
            nc.vector.memset(xsum[:], 0.0)

            tabs = [dram.tile([NPAD, H], f16, addr_space="Shared",
                               name=f"tab{i}") for i in range(NLAYERS)]
            ag_ins = [dram.tile([SH, H], f16, name=f"ag_in{i}")
                      for i in range(NLAYERS)]

            def write_table(hT_src, tab, ag_in):
                """node-major fp16 copy of this core's h shard -> AllGather."""
                hnm = hnmp.tile([P, TILES * P], f16, tag="hnm", name="hnm")
                for t in range(TILES):
                    blk16 = small.tile([P, P], f16, tag="blk16", name="blk16")
                    nc.scalar.activation(
                        out=blk16[:], in_=hT_src[:, t * P : (t + 1) * P],
                        func=AF.Copy)
                    pst = pstp.tile([P, P], f16, tag="tp", name="pst")
                    nc.tensor.transpose(out=pst[:], in_=blk16[:],
                                        identity=ident_sb[:])
                    if t == TILES - 1:
                        # zero the dummy-node rows (tail partitions) via mask
                        nc.scalar.activation(
                            out=hnm[:, t * P : (t + 1) * P], in_=pst[:],
                            func=AF.Copy, scale=dmask_sb[:, 0:1])
                    else:
                        nc.scalar.activation(
                            out=hnm[:, t * P : (t + 1) * P], in_=pst[:],
                            func=AF.Copy)
                nc.sync.dma_start(
                    out=ag_in[:].rearrange("(t p) f -> p t f", p=P),
                    in_=hnm[:].rearrange("p (t f) -> p t f", f=P))
                if not os.environ.get("GNN_SKIP_AG"):
                    nc.gpsimd.collective_compute(
                        "AllGather", mybir.AluOpType.bypass,
                        replica_groups=[list(range(NC))],
                        ins=[ag_in[:]], outs=[tab[:]])

            def gather_agg(layer, tab):
                """aggT fp16 [128 feat, SH nodes] for this layer."""
                aggT = aggTp.tile([P, SH], f16, tag="aggT", name="aggT")
                if os.environ.get("GNN_SKIP_GATHER"):
                    nc.vector.memset(aggT[:], 0.0)
                    return aggT
                for t in range(TILES):
                    da, db = int(DA[t]), int(DB[t])
                    ps = psagg.tile([P, P], f32, tag="agg", name="psa")
                    nblk = da + db
                    k = 0
                    if da:
                        gA = gAp.tile([P, da * P], f16, tag="gA", name="gA")
                        nc.gpsimd.dma_gather(
                            gA[:].rearrange("p (b e) -> p b e", e=H),
                            tab[:],
                            idxA_sb[:, baseA[t] // 16 : baseA[t] // 16 + da * 8],
                            da * P, da * P, H,
                            single_packet=False, queue_num=(2 * t) % NQ)
                        for d in range(da):
                            nc.tensor.matmul(
                                out=ps[:], lhsT=ident_sb[:],
                                rhs=gA[:, d * P : (d + 1) * P],
                                start=(k == 0), stop=(k == nblk - 1))
                            k += 1
                    if db:
                        gB = gBp.tile([P, db * P], f16, tag="gB", name="gB")
                        nc.gpsimd.dma_gather(
                            gB[:].rearrange("p (b e) -> p b e", e=H),
                            tab[B0:, :],
                            idxB_sb[:, baseB[t] // 16 : baseB[t] // 16 + db * 8],
                            db * P, db * P, H,
                            single_packet=False, queue_num=(2 * t + 1) % NQ)
                        for d in range(db):
                            nc.tensor.matmul(
                                out=ps[:], lhsT=ident_sb[:],
                                rhs=gB[:, d * P : (d + 1) * P],
                                start=(k == 0), stop=(k == nblk - 1))
                            k += 1
                    # deg_inv scale + cast to fp16 on the ACT engine
                    agg_nm = small.tile([P, P], f16, tag="aggnm", name="aggnm")
                    nc.scalar.activation(
                        out=agg_nm[:], in_=ps[:], func=AF.Copy,
                        scale=dinv_sb[:, t : t + 1])
                    pst = pstp.tile([P, P], f16, tag="tp", name="pst2")
                    nc.tensor.transpose(out=pst[:], in_=agg_nm[:],
                                        identity=ident_sb[:])
                    nc.scalar.activation(
                        out=aggT[:, t * P : (t + 1) * P], in_=pst[:],
                        func=AF.Copy)
                return aggT

            # ---- layer 0: h0 = x @ W_in + b_in ----
            hT = hTp.tile([P, SH], f32, tag="hT", name="hT0")
            for off, w in DENSE_BLOCKS:
                xblk = small.tile([P, 512], f32, tag="xblk", name="xblk", bufs=2)
                nc.sync.dma_start(out=xblk[:, :w], in_=xT[:, off : off + w])
                ps = psz.tile([P, 512], f32, tag="z", name="ps0")
                nc.tensor.matmul(out=ps[:, :w], lhsT=w_in_sb[:],
                                 rhs=xblk[:, :w],
                                 start=True, stop=True)
                nc.scalar.activation(out=hT[:, off : off + w], in_=ps[:, :w],
                                     func=AF.Identity, bias=b_in_sb[:, 0:1])
            write_table(hT, tabs[0], ag_ins[0])

            # ---- GNN layers ----
            for layer in range(NLAYERS):
                aggT = gather_agg(layer, tabs[layer])
                hT2 = hTp.tile([P, SH], f32, tag="hT", name=f"hT{layer + 1}")
                for off, w in DENSE_BLOCKS:
                    ps = psz.tile([P, 512], f32, tag="z", name=f"psz{layer}")
                    nc.tensor.matmul(
                        out=ps[:, :w], lhsT=wl_sb[:, layer * H : (layer + 1) * H],
                        rhs=hT[:, off : off + w], start=True, stop=False)
                    nc.tensor.matmul(
                        out=ps[:, :w], lhsT=wr_sb[:, layer * H : (layer + 1) * H],
                        rhs=aggT[:, off : off + w], start=False, stop=True,
                        skip_group_check=True)
                    nc.scalar.activation(
                        out=hT2[:, off : off + w], in_=ps[:, :w],
                        func=AF.Relu, bias=bl_sb[:, layer : layer + 1])
                nc.vector.tensor_add(out=xsum[:], in0=xsum[:], in1=hT2[:])
                if layer < NLAYERS - 1:
                    write_table(hT2, tabs[layer + 1], ag_ins[layer + 1])
                hT = hT2

            # ---- classifier ----
            outT_sb = big.tile([ODIM, SH], f32)
            for off, w in DENSE_BLOCKS:
                ps = psz.tile([P, 512], f32, tag="z", name="psc1")
                nc.tensor.matmul(out=ps[:, :w], lhsT=wc1_sb[:],
                                 rhs=xsum[:, off : off + w],
                                 start=True, stop=True)
                hc = small.tile([P, 512], f32, tag="hc", name="hc", bufs=2)
                nc.scalar.activation(out=hc[:, :w], in_=ps[:, :w],
                                     func=AF.Relu, bias=bc1_sb[:, 0:1])
                ps2 = psz.tile([ODIM, 512], f32, tag="z2", name="psc2", bufs=1)
                nc.tensor.matmul(out=ps2[:, :w], lhsT=wc2_sb[:],
                                 rhs=hc[:, :w], start=True, stop=True)
                nc.scalar.activation(out=outT_sb[:, off : off + w],
                                     in_=ps2[:, :w], func=AF.Identity,
                                     bias=bc2_sb[:, 0:1])
            nc.sync.dma_start(out=outT[:], in_=outT_sb[:])

    nc.compile()
    return nc


def kernel(x, edge_index, W_in, b_in, Wl, Wr, bl, Wc1, bc1, Wc2, bc2):
    global last_results
    from concourse.bass_utils import run_bass_kernel_spmd

    x = np.asarray(x, np.float32)
    edge_index = np.asarray(edge_index)
    meta = _prep(edge_index)
    DA, DB = meta["DA"], meta["DB"]
    baseA, baseB = meta["baseA"], meta["baseB"]
    CA, CB = int(baseA[-1]), int(baseB[-1])
    newid = meta["newid"]

    if NQ > 1:
        undo = _patch_swdge_lane_by_queue()
        try:
            nc = _build_program(DA, DB, baseA, baseB, CA, CB)
        finally:
            undo()
    else:
        nc = _build_program(DA, DB, baseA, baseB, CA, CB)

    # ---- per-core inputs ----
    dinv_full = np.ones(NC * SH, np.float32)
    dinv_full[newid] = meta["dinv"]
    x_full = np.zeros((NC * SH, H), np.float32)
    x_full[newid] = x

    ident = np.eye(P, dtype=np.float16)
    dmask = np.ones((P, 1), np.float32)
    dmask[P - N_DUMMY :] = 0.0
    wl_cat = np.concatenate([Wl[i] for i in range(NLAYERS)], 1).astype(np.float32)
    wr_cat = np.concatenate([Wr[i] for i in range(NLAYERS)], 1).astype(np.float16)
    bl_T = np.asarray(bl, np.float32).T.copy()  # [H, 3]

    in_maps = []
    for c in range(NC):
        sl = slice(c * SH, (c + 1) * SH)
        in_maps.append({
            "xT": x_full[sl].T.copy(),
            "idxA": np.tile(meta["arrA"][c], (8, 1)).astype(np.int16),
            "idxB": np.tile(meta["arrB"][c], (8, 1)).astype(np.int16),
            "dinv": dinv_full[sl].reshape(TILES, P).T.copy(),
            "w_in": np.asarray(W_in, np.float32),
            "b_in": np.asarray(b_in, np.float32).reshape(H, 1),
            "wl": wl_cat, "wr": wr_cat, "bl": bl_T,
            "wc1": np.asarray(Wc1, np.float32),
            "bc1": np.asarray(bc1, np.float32).reshape(H, 1),
            "wc2": np.asarray(Wc2, np.float32),
            "bc2": np.asarray(bc2, np.float32).reshape(ODIM, 1),
            "ident": ident,
            "dmask": dmask,
        })

    trace = bool(int(os.environ.get("GNN_TRACE", "0")))
    res = run_bass_kernel_spmd(nc, in_maps, list(range(NC)), trace=trace)
    last_results = res

    big = np.concatenate([res.results[c]["outT"] for c in range(NC)], axis=1)
    out = big.T[newid]  # [N, ODIM]
    return out.astype(np.float32)

